# revision 6
# baseline (speedup 1.0000x reference)
"""Trainium2 Bass kernel for nn_AtomFeature (retrieval_knn).

Problem: B=2, N=4608 atoms, 3D coords. Outputs:
  atom_embedding (B,N,32)  - graph-normed tiled embedding table
  cross_dists    (B,N,32)  - distances to K=32 nearest neighbors
  edge_index     (B,N,32)  - indices of those neighbors (int32)

Sharding: the B*N = 9216 query rows are split across 8 cores (1152 rows
each; cores 0-3 handle batch 0, cores 4-7 batch 1). Each core receives
the full 4608 key coords of its batch (replicated) - no collectives.

Per 128-query tile (9 per core):
  ScalarE : t_c = Square(key_c_row - q_c)  (bit-exact, verified on HW)
  GpSimd  : nd = -(t0+t1) - t2 = -d^2      (same rounding as reference)
  VectorE : 4 rounds of max8 / max_index / match_replace -> exact top-32
            of -d^2 with jax.lax.top_k's lowest-index-first tie handling
  ScalarE+VectorE: dist = sqrt(d^2+1e-6) via LUT sqrt + 2 Newton steps
The embedding branch reduces the mask to 12 per-atom-type counts (DVE),
computes graph-norm stats from the 12x32 table, and applies the affine
per tile.
"""
import numpy as np

B = 2
N = 4608
D = 32
K = 32
NTYPES = 12
NCORES = 8
ROWS_PER_CORE = (B * N) // NCORES  # 1152
NTILES = ROWS_PER_CORE // 128      # 9
BIG = 1000000.0
EPS_NORM = 1e-5
EPS_DIST = 1e-6
NEG_FILL = -1.0e30

_compiled = None


def _build():
    import concourse.bacc as bacc
    from concourse import mybir
    from concourse.tile import TileContext

    f32 = mybir.dt.float32
    u32 = mybir.dt.uint32
    i32 = mybir.dt.int32
    Alu = mybir.AluOpType
    Act = mybir.ActivationFunctionType

    nc = bacc.Bacc(None, target_bir_lowering=False, debug=False)

    qrows_ext = nc.declare_dram_parameter("qrows", [ROWS_PER_CORE, 3], f32, isOutput=False)
    keysT_ext = nc.declare_dram_parameter("keysT", [3, N], f32, isOutput=False)
    maskr_ext = nc.declare_dram_parameter("maskr", [ROWS_PER_CORE, 1], f32, isOutput=False)
    maskf_ext = nc.declare_dram_parameter("maskf", [1, N], f32, isOutput=False)
    embrep_ext = nc.declare_dram_parameter("embrep", [144, D], f32, isOutput=False)
    etabT_ext = nc.declare_dram_parameter("etabT", [D, NTYPES], f32, isOutput=False)
    scale_ext = nc.declare_dram_parameter("scalecol", [D, 1], f32, isOutput=False)
    shift_ext = nc.declare_dram_parameter("shiftcol", [D, 1], f32, isOutput=False)

    emb_out = nc.declare_dram_parameter("emb_out", [ROWS_PER_CORE, D], f32, isOutput=True)
    dist_out = nc.declare_dram_parameter("dist_out", [ROWS_PER_CORE, K], f32, isOutput=True)
    d2_out = nc.declare_dram_parameter("d2_out", [ROWS_PER_CORE, K], f32, isOutput=True)
    idx_out = nc.declare_dram_parameter("idx_out", [ROWS_PER_CORE, K], i32, isOutput=True)

    arow_dram = nc.dram_tensor("arow_dram", [D, 2], f32)

    with TileContext(nc) as tc:
        with (
            tc.tile_pool(name="persist", bufs=1) as pp,
            tc.tile_pool(name="work", bufs=1) as wp,
            tc.tile_pool(name="ndpool", bufs=2) as np_pool,
            tc.tile_pool(name="small", bufs=2) as sp,
        ):
            # ---- key coordinate planes, replicated to 128 partitions ----
            kx = pp.tile([128, N], f32)
            ky = pp.tile([128, N], f32)
            kz = pp.tile([128, N], f32)
            nc.sync.dma_start(out=kx[:, :], in_=keysT_ext[0:1, :].partition_broadcast(128))
            nc.sync.dma_start(out=ky[:, :], in_=keysT_ext[1:2, :].partition_broadcast(128))
            nc.sync.dma_start(out=kz[:, :], in_=keysT_ext[2:3, :].partition_broadcast(128))

            # ---- graph-norm statistics from per-type mask counts ----
            mf = pp.tile([1, N], f32)
            nc.sync.dma_start(out=mf[:, :], in_=maskf_ext[:, :])
            etabT = pp.tile([D, NTYPES], f32)
            nc.sync.dma_start(out=etabT[:, :], in_=etabT_ext[:, :])
            scol = pp.tile([D, 1], f32)
            nc.sync.dma_start(out=scol[:, :], in_=scale_ext[:, :])
            shcol = pp.tile([D, 1], f32)
            nc.sync.dma_start(out=shcol[:, :], in_=shift_ext[:, :])

            ts = pp.tile([1, NTYPES], f32)
            # mask[n], n = g*12 + r  ->  ts[r] = sum_g mask[g*12+r]
            nc.vector.reduce_sum(ts[:, :], mf[0:1, :].rearrange("p (g r) -> p r g", r=NTYPES),
                                 axis=mybir.AxisListType.X)
            cnt_raw = pp.tile([1, 1], f32)
            nc.vector.reduce_sum(cnt_raw[:, :], ts[:, :], axis=mybir.AxisListType.X)
            cnt1 = pp.tile([1, 1], f32)
            nc.vector.tensor_scalar_max(cnt1[:, :], cnt_raw[:, :], 1.0)
            rc = pp.tile([1, 1], f32)
            nc.vector.reciprocal(rc[:, :], cnt1[:, :])
            nmc = pp.tile([1, 1], f32)  # N - sum(mask)
            nc.vector.tensor_scalar(nmc[:, :], cnt_raw[:, :], -1.0, float(N), Alu.mult, Alu.add)

            tsb = pp.tile([D, NTYPES], f32)
            nc.gpsimd.partition_broadcast(tsb[:, :], ts[:, :])
            rcb = pp.tile([D, 1], f32)
            nc.gpsimd.partition_broadcast(rcb[:, :], rc[:, :])
            nmcb = pp.tile([D, 1], f32)
            nc.gpsimd.partition_broadcast(nmcb[:, :], nmc[:, :])

            tmp = pp.tile([D, NTYPES], f32)
            nc.vector.tensor_tensor(tmp[:, :], etabT[:, :], tsb[:, :], Alu.mult)
            meanT = pp.tile([D, 1], f32)
            nc.vector.reduce_sum(meanT[:, :], tmp[:, :], axis=mybir.AxisListType.X)
            nc.vector.tensor_scalar(meanT[:, :], meanT[:, :], rcb[:, 0:1], None, Alu.mult)
            negmeanT = pp.tile([D, 1], f32)
            nc.vector.tensor_scalar_mul(negmeanT[:, :], meanT[:, :], -1.0)

            sqT = pp.tile([D, NTYPES], f32)
            nc.scalar.activation(sqT[:, :], etabT[:, :], Act.Square, bias=negmeanT[:, 0:1], scale=1.0)
            nc.vector.tensor_tensor(sqT[:, :], sqT[:, :], tsb[:, :], Alu.mult)
            varT = pp.tile([D, 1], f32)
            nc.vector.reduce_sum(varT[:, :], sqT[:, :], axis=mybir.AxisListType.X)
            msq = pp.tile([D, 1], f32)
            nc.vector.tensor_tensor(msq[:, :], meanT[:, :], meanT[:, :], Alu.mult)
            nc.vector.tensor_scalar(msq[:, :], msq[:, :], nmcb[:, 0:1], None, Alu.mult)
            nc.vector.tensor_tensor(varT[:, :], varT[:, :], msq[:, :], Alu.add)
            nc.vector.tensor_scalar(varT[:, :], varT[:, :], rcb[:, 0:1], EPS_NORM, Alu.mult, Alu.add)

            # std = sqrt(varT) with 2 Newton refinements of the LUT sqrt
            stdT = pp.tile([D, 1], f32)
            nc.scalar.activation(stdT[:, :], varT[:, :], Act.Sqrt)
            for _ in range(2):
                r_ = pp.tile([D, 1], f32, tag="newt_r")
                nc.vector.reciprocal(r_[:, :], stdT[:, :])
                nc.vector.tensor_tensor(r_[:, :], varT[:, :], r_[:, :], Alu.mult)
                nc.vector.tensor_tensor(stdT[:, :], stdT[:, :], r_[:, :], Alu.add)
                nc.vector.tensor_scalar_mul(stdT[:, :], stdT[:, :], 0.5)
            rstdT = pp.tile([D, 1], f32)
            nc.vector.reciprocal(rstdT[:, :], stdT[:, :])

            a0T = pp.tile([D, 1], f32)
            nc.vector.tensor_tensor(a0T[:, :], rstdT[:, :], scol[:, :], Alu.mult)
            a1T = pp.tile([D, 1], f32)
            nc.vector.tensor_tensor(a1T[:, :], meanT[:, :], a0T[:, :], Alu.mult)
            nc.vector.tensor_tensor(a1T[:, :], shcol[:, :], a1T[:, :], Alu.subtract)

            # (D,1) columns -> (1,D) rows via DRAM bounce, then broadcast
            nc.sync.dma_start(out=arow_dram[:, 0:1], in_=a0T[:, :])
            nc.sync.dma_start(out=arow_dram[:, 1:2], in_=a1T[:, :])
            a0row = pp.tile([1, D], f32)
            nc.sync.dma_start(out=a0row[:, :], in_=arow_dram[:, 0:1])
            a1row = pp.tile([1, D], f32)
            nc.sync.dma_start(out=a1row[:, :], in_=arow_dram[:, 1:2])
            a0full = pp.tile([128, D], f32)
            nc.gpsimd.partition_broadcast(a0full[:, :], a0row[:, :])
            a1full = pp.tile([128, D], f32)
            nc.gpsimd.partition_broadcast(a1full[:, :], a1row[:, :])

            scratch = pp.tile([128, N], f32)
            zplane = pp.tile([128, N], f32)
            nc.gpsimd.memset(zplane[:, :], 0.0)

            # ---- main per-tile loop ----
            for t in range(NTILES):
                lo = t * 128
                off = (t * 128) % NTYPES  # 0, 8, 4, ...

                qt = sp.tile([128, 3], f32)
                nc.sync.dma_start(out=qt[:, :], in_=qrows_ext[lo:lo + 128, :])
                nqt = sp.tile([128, 3], f32)
                nc.vector.tensor_scalar_mul(nqt[:, :], qt[:, :], -1.0)
                mt = sp.tile([128, 1], f32)
                nc.sync.dma_start(out=mt[:, :], in_=maskr_ext[lo:lo + 128, :])
                et = sp.tile([128, D], f32)
                nc.sync.dma_start(out=et[:, :], in_=embrep_ext[off:off + 128, :])

                # squared coordinate deltas (ScalarE, bit-exact)
                t0 = wp.tile([128, N], f32)
                nc.scalar.activation(t0[:, :], kx[:, :], Act.Square, bias=nqt[:, 0:1], scale=1.0)
                t1 = wp.tile([128, N], f32)
                nc.scalar.activation(t1[:, :], ky[:, :], Act.Square, bias=nqt[:, 1:2], scale=1.0)
                t2 = wp.tile([128, N], f32)
                nc.scalar.activation(t2[:, :], kz[:, :], Act.Square, bias=nqt[:, 2:3], scale=1.0)

                # nd = (0 - (t0+t1)) - t2 == -(d^2), same rounding as reference
                nd = np_pool.tile([128, N], f32, tag="nd")
                nc.gpsimd.tensor_tensor(nd[:, :], t0[:, :], t1[:, :], Alu.add)
                nc.gpsimd.tensor_tensor(nd[:, :], zplane[:, :], nd[:, :], Alu.subtract)
                nc.gpsimd.tensor_tensor(nd[:, :], nd[:, :], t2[:, :], Alu.subtract)

                # exact top-32 of nd (descending) == top-32 smallest d^2
                vals = sp.tile([128, K], f32)
                idxu = sp.tile([128, K], u32)
                cur, alt = nd, scratch
                for r in range(4):
                    v8 = vals[:, 8 * r:8 * r + 8]
                    i8 = idxu[:, 8 * r:8 * r + 8]
                    nc.vector.max(v8, cur[:, :])
                    nc.vector.max_index(i8, v8, cur[:, :])
                    if r < 3:
                        nc.vector.match_replace(alt[:, :], v8, cur[:, :], NEG_FILL)
                        cur, alt = alt, cur

                # d2 and dist = sqrt(d2 + 1e-6) (LUT sqrt + 2 Newton steps)
                d2 = sp.tile([128, K], f32)
                nc.vector.tensor_scalar_mul(d2[:, :], vals[:, :], -1.0)
                x32 = sp.tile([128, K], f32)
                nc.vector.tensor_scalar_add(x32[:, :], d2[:, :], EPS_DIST)
                # seed = sqrt(x*1e6)*1e-3: keeps the LUT in its accurate
                # range (it returns 0 for x ~ 1e-6); Newton polishes on x.
                y = sp.tile([128, K], f32)
                nc.scalar.activation(y[:, :], x32[:, :], Act.Sqrt, scale=1.0e6)
                nc.vector.tensor_scalar_mul(y[:, :], y[:, :], 1.0e-3)
                for _ in range(2):
                    rn = sp.tile([128, K], f32, tag="rn")
                    nc.vector.reciprocal(rn[:, :], y[:, :])
                    nc.vector.tensor_tensor(rn[:, :], x32[:, :], rn[:, :], Alu.mult)
                    nc.vector.tensor_tensor(y[:, :], y[:, :], rn[:, :], Alu.add)
                    nc.vector.tensor_scalar_mul(y[:, :], y[:, :], 0.5)

                # pad handling: dist -> BIG, idx -> -1 where mask == 0
                # (cancellation-free: y*m + BIG*(1-m))
                bw = sp.tile([128, 1], f32)
                nc.vector.tensor_scalar(bw[:, :], mt[:, :], -BIG, BIG, Alu.mult, Alu.add)
                distf = sp.tile([128, K], f32)
                nc.vector.tensor_scalar(distf[:, :], y[:, :], mt[:, 0:1], bw[:, 0:1], Alu.mult, Alu.add)
                idxf = sp.tile([128, K], f32)
                nc.vector.tensor_copy(idxf[:, :], idxu[:, :])
                nc.vector.tensor_scalar(idxf[:, :], idxf[:, :], 1.0, mt[:, 0:1], Alu.add, Alu.mult)
                nc.vector.tensor_scalar_add(idxf[:, :], idxf[:, :], -1.0)
                idxi = sp.tile([128, K], i32)
                nc.vector.tensor_copy(idxi[:, :], idxf[:, :])

                # embedding: (E*a0 + a1) * mask
                z = sp.tile([128, D], f32)
                nc.vector.tensor_tensor(z[:, :], et[:, :], a0full[:, :], Alu.mult)
                nc.vector.tensor_tensor(z[:, :], z[:, :], a1full[:, :], Alu.add)
                nc.vector.tensor_scalar(z[:, :], z[:, :], mt[:, 0:1], None, Alu.mult)

                nc.sync.dma_start(out=emb_out[lo:lo + 128, :], in_=z[:, :])
                nc.sync.dma_start(out=dist_out[lo:lo + 128, :], in_=distf[:, :])
                nc.sync.dma_start(out=d2_out[lo:lo + 128, :], in_=d2[:, :])
                nc.sync.dma_start(out=idx_out[lo:lo + 128, :], in_=idxi[:, :])

    nc.compile()
    return nc


def _get_compiled():
    global _compiled
    if _compiled is None:
        _compiled = _build()
    return _compiled


def kernel(atom_coords, atom_mask, emb_table, scale, shift):
    from concourse.bass_utils import run_bass_kernel_spmd

    nc = _get_compiled()

    atom_coords = np.asarray(atom_coords, dtype=np.float32)
    atom_mask = np.asarray(atom_mask, dtype=np.float32)
    emb_table = np.asarray(emb_table, dtype=np.float32)
    scale = np.asarray(scale, dtype=np.float32).reshape(D, 1)
    shift = np.asarray(shift, dtype=np.float32).reshape(D, 1)

    embrep = np.ascontiguousarray(np.tile(emb_table, (12, 1)))  # (144, D)
    etabT = np.ascontiguousarray(emb_table.T)                    # (D, 12)

    in_maps = []
    for c in range(NCORES):
        b = c // (NCORES // B)
        lo = (c % (NCORES // B)) * ROWS_PER_CORE
        in_maps.append({
            "qrows": np.ascontiguousarray(atom_coords[b, lo:lo + ROWS_PER_CORE, :]),
            "keysT": np.ascontiguousarray(atom_coords[b].T),
            "maskr": np.ascontiguousarray(atom_mask[b, lo:lo + ROWS_PER_CORE, None]),
            "maskf": np.ascontiguousarray(atom_mask[b][None, :]),
            "embrep": embrep,
            "etabT": etabT,
            "scalecol": scale,
            "shiftcol": shift,
        })

    res = run_bass_kernel_spmd(nc, in_maps, core_ids=list(range(NCORES)))

    emb = np.concatenate([res.results[c]["emb_out"] for c in range(NCORES)], axis=0)
    dist = np.concatenate([res.results[c]["dist_out"] for c in range(NCORES)], axis=0)
    d2 = np.concatenate([res.results[c]["d2_out"] for c in range(NCORES)], axis=0)
    idx = np.concatenate([res.results[c]["idx_out"] for c in range(NCORES)], axis=0)

    emb = emb.reshape(B, N, D)
    dist = dist.reshape(B, N, K)
    d2 = d2.reshape(B, N, K)
    idx = idx.reshape(B, N, K)

    # Tie-order fixup: the device selects by d^2; the reference sorts by
    # dist = sqrt(d^2+1e-6), breaking ties by lower index. Two distinct d^2
    # can round to the same f32 dist - reorder indices inside equal-dist
    # runs to ascending, matching jax.lax.top_k.
    dist_h = np.sqrt(d2 + np.float32(EPS_DIST), dtype=np.float32)
    ties = dist_h[:, :, 1:] == dist_h[:, :, :-1]
    if ties.any():
        rows = np.argwhere(ties.any(axis=2))
        valid = atom_mask > 0
        for bb, nn_ in rows:
            if not valid[bb, nn_]:
                continue
            row_d = dist_h[bb, nn_]
            row_i = idx[bb, nn_]
            s = 0
            while s < K:
                e = s + 1
                while e < K and row_d[e] == row_d[s]:
                    e += 1
                if e - s > 1:
                    row_i[s:e] = np.sort(row_i[s:e])
                s = e
            idx[bb, nn_] = row_i

    return emb, dist, idx.astype(np.int32)


# revision 14
# speedup vs baseline: 1.3667x; 1.3667x over previous
"""Trainium2 Bass kernel for nn_AtomFeature (retrieval_knn).

Problem: B=2, N=4608 atoms, 3D coords. Outputs:
  atom_embedding (B,N,32)  - graph-normed tiled embedding table
  cross_dists    (B,N,32)  - distances to K=32 nearest neighbors
  edge_index     (B,N,32)  - indices of those neighbors (int32)

Sharding: the B*N = 9216 query rows are split across 8 cores (1152 rows
each; cores 0-3 handle batch 0, cores 4-7 batch 1). Each core receives
the full 4608 key coords of its batch (replicated) - no collectives.

Per 128-query tile (9 per core):
  ScalarE : t_c = Square(key_c_row - q_c)  (bit-exact, verified on HW)
  GpSimd  : nd = -(t0+t1) - t2 = -d^2      (same rounding as reference)
  VectorE : 4 rounds of max8 / max_index / match_replace -> exact top-32
            of -d^2 with jax.lax.top_k's lowest-index-first tie handling
  ScalarE+VectorE: dist = sqrt(d^2+1e-6) via LUT sqrt + 2 Newton steps
The embedding branch reduces the mask to 12 per-atom-type counts (DVE),
computes graph-norm stats from the 12x32 table, and applies the affine
per tile.
"""
import numpy as np

B = 2
N = 4608
D = 32
K = 32
NTYPES = 12
NCORES = 8
ROWS_PER_CORE = (B * N) // NCORES  # 1152
NTILES = ROWS_PER_CORE // 128      # 9
BIG = 1000000.0
EPS_NORM = 1e-5
EPS_DIST = 1e-6
NEG_FILL = -1.0e30

_compiled = None


def _build():
    import concourse.bacc as bacc
    from concourse import mybir
    from concourse.tile import TileContext

    f32 = mybir.dt.float32
    u32 = mybir.dt.uint32
    i32 = mybir.dt.int32
    Alu = mybir.AluOpType
    Act = mybir.ActivationFunctionType

    nc = bacc.Bacc(None, target_bir_lowering=False, debug=False)

    qrows_ext = nc.declare_dram_parameter("qrows", [ROWS_PER_CORE, 3], f32, isOutput=False)
    keysT_ext = nc.declare_dram_parameter("keysT", [3, N], f32, isOutput=False)
    maskr_ext = nc.declare_dram_parameter("maskr", [ROWS_PER_CORE, 1], f32, isOutput=False)
    maskf_ext = nc.declare_dram_parameter("maskf", [1, N], f32, isOutput=False)
    embrep_ext = nc.declare_dram_parameter("embrep", [144, D], f32, isOutput=False)
    etabT_ext = nc.declare_dram_parameter("etabT", [D, NTYPES], f32, isOutput=False)
    scale_ext = nc.declare_dram_parameter("scalecol", [D, 1], f32, isOutput=False)
    shift_ext = nc.declare_dram_parameter("shiftcol", [D, 1], f32, isOutput=False)

    emb_out = nc.declare_dram_parameter("emb_out", [ROWS_PER_CORE, D], f32, isOutput=True)
    dist_out = nc.declare_dram_parameter("dist_out", [ROWS_PER_CORE, K], f32, isOutput=True)
    d2_out = nc.declare_dram_parameter("d2_out", [ROWS_PER_CORE, K], f32, isOutput=True)
    idx_out = nc.declare_dram_parameter("idx_out", [ROWS_PER_CORE, K], i32, isOutput=True)

    arow_dram = nc.dram_tensor("arow_dram", [D, 2], f32)

    with TileContext(nc) as tc:
        with (
            tc.tile_pool(name="persist", bufs=1) as pp,
            tc.tile_pool(name="work", bufs=1) as wp,
            tc.tile_pool(name="ndpool", bufs=2) as np_pool,
            tc.tile_pool(name="small", bufs=2) as sp,
        ):
            # ---- key coordinate planes, replicated to 128 partitions ----
            kx = pp.tile([128, N], f32)
            ky = pp.tile([128, N], f32)
            kz = pp.tile([128, N], f32)
            nc.sync.dma_start(out=kx[:, :], in_=keysT_ext[0:1, :].partition_broadcast(128))
            nc.sync.dma_start(out=ky[:, :], in_=keysT_ext[1:2, :].partition_broadcast(128))
            nc.sync.dma_start(out=kz[:, :], in_=keysT_ext[2:3, :].partition_broadcast(128))

            # ---- graph-norm statistics from per-type mask counts ----
            mf = pp.tile([1, N], f32)
            nc.sync.dma_start(out=mf[:, :], in_=maskf_ext[:, :])
            etabT = pp.tile([D, NTYPES], f32)
            nc.sync.dma_start(out=etabT[:, :], in_=etabT_ext[:, :])
            scol = pp.tile([D, 1], f32)
            nc.sync.dma_start(out=scol[:, :], in_=scale_ext[:, :])
            shcol = pp.tile([D, 1], f32)
            nc.sync.dma_start(out=shcol[:, :], in_=shift_ext[:, :])

            ts = pp.tile([1, NTYPES], f32)
            # mask[n], n = g*12 + r  ->  ts[r] = sum_g mask[g*12+r]
            nc.vector.reduce_sum(ts[:, :], mf[0:1, :].rearrange("p (g r) -> p r g", r=NTYPES),
                                 axis=mybir.AxisListType.X)
            cnt_raw = pp.tile([1, 1], f32)
            nc.vector.reduce_sum(cnt_raw[:, :], ts[:, :], axis=mybir.AxisListType.X)
            cnt1 = pp.tile([1, 1], f32)
            nc.vector.tensor_scalar_max(cnt1[:, :], cnt_raw[:, :], 1.0)
            rc = pp.tile([1, 1], f32)
            nc.vector.reciprocal(rc[:, :], cnt1[:, :])
            nmc = pp.tile([1, 1], f32)  # N - sum(mask)
            nc.vector.tensor_scalar(nmc[:, :], cnt_raw[:, :], -1.0, float(N), Alu.mult, Alu.add)

            tsb = pp.tile([D, NTYPES], f32)
            nc.gpsimd.partition_broadcast(tsb[:, :], ts[:, :])
            rcb = pp.tile([D, 1], f32)
            nc.gpsimd.partition_broadcast(rcb[:, :], rc[:, :])
            nmcb = pp.tile([D, 1], f32)
            nc.gpsimd.partition_broadcast(nmcb[:, :], nmc[:, :])

            tmp = pp.tile([D, NTYPES], f32)
            nc.vector.tensor_tensor(tmp[:, :], etabT[:, :], tsb[:, :], Alu.mult)
            meanT = pp.tile([D, 1], f32)
            nc.vector.reduce_sum(meanT[:, :], tmp[:, :], axis=mybir.AxisListType.X)
            nc.vector.tensor_scalar(meanT[:, :], meanT[:, :], rcb[:, 0:1], None, Alu.mult)
            negmeanT = pp.tile([D, 1], f32)
            nc.vector.tensor_scalar_mul(negmeanT[:, :], meanT[:, :], -1.0)

            sqT = pp.tile([D, NTYPES], f32)
            nc.scalar.activation(sqT[:, :], etabT[:, :], Act.Square, bias=negmeanT[:, 0:1], scale=1.0)
            nc.vector.tensor_tensor(sqT[:, :], sqT[:, :], tsb[:, :], Alu.mult)
            varT = pp.tile([D, 1], f32)
            nc.vector.reduce_sum(varT[:, :], sqT[:, :], axis=mybir.AxisListType.X)
            msq = pp.tile([D, 1], f32)
            nc.vector.tensor_tensor(msq[:, :], meanT[:, :], meanT[:, :], Alu.mult)
            nc.vector.tensor_scalar(msq[:, :], msq[:, :], nmcb[:, 0:1], None, Alu.mult)
            nc.vector.tensor_tensor(varT[:, :], varT[:, :], msq[:, :], Alu.add)
            nc.vector.tensor_scalar(varT[:, :], varT[:, :], rcb[:, 0:1], EPS_NORM, Alu.mult, Alu.add)

            # std = sqrt(varT) with 2 Newton refinements of the LUT sqrt
            stdT = pp.tile([D, 1], f32)
            nc.scalar.activation(stdT[:, :], varT[:, :], Act.Sqrt)
            for _ in range(2):
                r_ = pp.tile([D, 1], f32, tag="newt_r")
                nc.vector.reciprocal(r_[:, :], stdT[:, :])
                nc.vector.tensor_tensor(r_[:, :], varT[:, :], r_[:, :], Alu.mult)
                nc.vector.tensor_tensor(stdT[:, :], stdT[:, :], r_[:, :], Alu.add)
                nc.vector.tensor_scalar_mul(stdT[:, :], stdT[:, :], 0.5)
            rstdT = pp.tile([D, 1], f32)
            nc.vector.reciprocal(rstdT[:, :], stdT[:, :])

            a0T = pp.tile([D, 1], f32)
            nc.vector.tensor_tensor(a0T[:, :], rstdT[:, :], scol[:, :], Alu.mult)
            a1T = pp.tile([D, 1], f32)
            nc.vector.tensor_tensor(a1T[:, :], meanT[:, :], a0T[:, :], Alu.mult)
            nc.vector.tensor_tensor(a1T[:, :], shcol[:, :], a1T[:, :], Alu.subtract)

            # (D,1) columns -> (1,D) rows via DRAM bounce, then broadcast
            nc.sync.dma_start(out=arow_dram[:, 0:1], in_=a0T[:, :])
            nc.sync.dma_start(out=arow_dram[:, 1:2], in_=a1T[:, :])
            a0row = pp.tile([1, D], f32)
            nc.sync.dma_start(out=a0row[:, :], in_=arow_dram[:, 0:1])
            a1row = pp.tile([1, D], f32)
            nc.sync.dma_start(out=a1row[:, :], in_=arow_dram[:, 1:2])
            a0full = pp.tile([128, D], f32)
            nc.gpsimd.partition_broadcast(a0full[:, :], a0row[:, :])
            a1full = pp.tile([128, D], f32)
            nc.gpsimd.partition_broadcast(a1full[:, :], a1row[:, :])

            scratch = pp.tile([128, N], f32)

            # constant bias columns for ScalarE activations
            c_eps = pp.tile([128, 1], f32)
            nc.gpsimd.memset(c_eps[:, :], EPS_DIST)
            c_big = pp.tile([128, 1], f32)
            nc.gpsimd.memset(c_big[:, :], BIG)
            c_neg1 = pp.tile([128, 1], f32)
            nc.gpsimd.memset(c_neg1[:, :], -1.0)

            # ---- main per-tile loop ----
            for t in range(NTILES):
                lo = t * 128
                off = (t * 128) % NTYPES  # 0, 8, 4, ...

                qt = sp.tile([128, 3], f32)
                nc.sync.dma_start(out=qt[:, :], in_=qrows_ext[lo:lo + 128, :])
                nqt = sp.tile([128, 3], f32)
                nc.scalar.mul(nqt[:, :], qt[:, :], -1.0)
                mt = sp.tile([128, 1], f32)
                nc.sync.dma_start(out=mt[:, :], in_=maskr_ext[lo:lo + 128, :])
                et = sp.tile([128, D], f32)
                nc.sync.dma_start(out=et[:, :], in_=embrep_ext[off:off + 128, :])

                # squared coordinate deltas (ScalarE, bit-exact)
                t0 = wp.tile([128, N], f32)
                nc.scalar.activation(t0[:, :], kx[:, :], Act.Square, bias=nqt[:, 0:1], scale=1.0)
                t1 = wp.tile([128, N], f32)
                nc.scalar.activation(t1[:, :], ky[:, :], Act.Square, bias=nqt[:, 1:2], scale=1.0)
                t2 = wp.tile([128, N], f32)
                nc.scalar.activation(t2[:, :], kz[:, :], Act.Square, bias=nqt[:, 2:3], scale=1.0)
                t2n = wp.tile([128, N], f32)
                nc.scalar.mul(t2n[:, :], t2[:, :], -1.0)

                # nd = (-t2) - (t0+t1) == -((t0+t1)+t2) bit-exactly (IEEE add
                # is commutative), matching the reference's d^2 rounding.
                nd = np_pool.tile([128, N], f32, tag="nd")
                nc.gpsimd.tensor_tensor(nd[:, :], t0[:, :], t1[:, :], Alu.add)
                nc.gpsimd.tensor_tensor(nd[:, :], t2n[:, :], nd[:, :], Alu.subtract)

                # exact top-32 of nd (descending) == top-32 smallest d^2
                vals = sp.tile([128, K], f32)
                idxu = sp.tile([128, K], u32)
                cur, alt = nd, scratch
                for r in range(4):
                    v8 = vals[:, 8 * r:8 * r + 8]
                    i8 = idxu[:, 8 * r:8 * r + 8]
                    nc.vector.max(v8, cur[:, :])
                    nc.vector.max_index(i8, v8, cur[:, :])
                    if r < 3:
                        nc.vector.match_replace(alt[:, :], v8, cur[:, :], NEG_FILL)
                        cur, alt = alt, cur

                # d2 and dist = sqrt(d2 + 1e-6); seed = sqrt((d2+eps)*1e6)*1e-3
                # (keeps the LUT in its accurate range), then 2 Newton steps.
                # Everything here runs on ScalarE/GpSimd so DVE stays on the
                # top-k scans; DVE only does the (cheap) reciprocals.
                d2 = sp.tile([128, K], f32)
                nc.scalar.mul(d2[:, :], vals[:, :], -1.0)
                x32 = sp.tile([128, K], f32)
                nc.scalar.activation(x32[:, :], vals[:, :], Act.Identity, bias=c_eps[:, 0:1], scale=-1.0)
                y = sp.tile([128, K], f32)
                nc.scalar.activation(y[:, :], vals[:, :], Act.Sqrt, bias=1.0, scale=-1.0e6)
                nc.scalar.mul(y[:, :], y[:, :], 1.0e-3)
                for _ in range(2):
                    rn = sp.tile([128, K], f32, tag="rn")
                    nc.vector.reciprocal(rn[:, :], y[:, :])
                    nc.gpsimd.tensor_tensor(rn[:, :], x32[:, :], rn[:, :], Alu.mult)
                    nc.gpsimd.tensor_tensor(y[:, :], y[:, :], rn[:, :], Alu.add)
                    nc.scalar.mul(y[:, :], y[:, :], 0.5)

                # pad handling: dist -> BIG, idx -> -1 where mask == 0
                # (cancellation-free: y*m + BIG*(1-m))
                bw = sp.tile([128, 1], f32)
                nc.scalar.activation(bw[:, :], mt[:, :], Act.Identity, bias=c_big[:, 0:1], scale=-BIG)
                distf = sp.tile([128, K], f32)
                nc.scalar.activation(distf[:, :], y[:, :], Act.Identity,
                                     bias=bw[:, 0:1], scale=mt[:, 0:1])
                idxf = sp.tile([128, K], f32)
                nc.scalar.activation(idxf[:, :], idxu[:, :], Act.Identity, bias=1.0, scale=1.0)
                idxm = sp.tile([128, K], f32)
                nc.scalar.activation(idxm[:, :], idxf[:, :], Act.Identity,
                                     bias=c_neg1[:, 0:1], scale=mt[:, 0:1])
                idxi = sp.tile([128, K], i32)
                nc.scalar.copy(idxi[:, :], idxm[:, :])

                # embedding: (E*a0 + a1) * mask
                z = sp.tile([128, D], f32)
                nc.gpsimd.tensor_tensor(z[:, :], et[:, :], a0full[:, :], Alu.mult)
                nc.gpsimd.tensor_tensor(z[:, :], z[:, :], a1full[:, :], Alu.add)
                nc.scalar.activation(z[:, :], z[:, :], Act.Identity, bias=0.0, scale=mt[:, 0:1])

                nc.sync.dma_start(out=emb_out[lo:lo + 128, :], in_=z[:, :])
                nc.sync.dma_start(out=dist_out[lo:lo + 128, :], in_=distf[:, :])
                nc.sync.dma_start(out=d2_out[lo:lo + 128, :], in_=d2[:, :])
                nc.sync.dma_start(out=idx_out[lo:lo + 128, :], in_=idxi[:, :])

    nc.compile()
    return nc


def _get_compiled():
    global _compiled
    if _compiled is None:
        _compiled = _build()
    return _compiled


def kernel(atom_coords, atom_mask, emb_table, scale, shift):
    from concourse.bass_utils import run_bass_kernel_spmd

    nc = _get_compiled()

    atom_coords = np.asarray(atom_coords, dtype=np.float32)
    atom_mask = np.asarray(atom_mask, dtype=np.float32)
    emb_table = np.asarray(emb_table, dtype=np.float32)
    scale = np.asarray(scale, dtype=np.float32).reshape(D, 1)
    shift = np.asarray(shift, dtype=np.float32).reshape(D, 1)

    embrep = np.ascontiguousarray(np.tile(emb_table, (12, 1)))  # (144, D)
    etabT = np.ascontiguousarray(emb_table.T)                    # (D, 12)

    in_maps = []
    for c in range(NCORES):
        b = c // (NCORES // B)
        lo = (c % (NCORES // B)) * ROWS_PER_CORE
        in_maps.append({
            "qrows": np.ascontiguousarray(atom_coords[b, lo:lo + ROWS_PER_CORE, :]),
            "keysT": np.ascontiguousarray(atom_coords[b].T),
            "maskr": np.ascontiguousarray(atom_mask[b, lo:lo + ROWS_PER_CORE, None]),
            "maskf": np.ascontiguousarray(atom_mask[b][None, :]),
            "embrep": embrep,
            "etabT": etabT,
            "scalecol": scale,
            "shiftcol": shift,
        })

    res = run_bass_kernel_spmd(nc, in_maps, core_ids=list(range(NCORES)))

    emb = np.concatenate([res.results[c]["emb_out"] for c in range(NCORES)], axis=0)
    dist = np.concatenate([res.results[c]["dist_out"] for c in range(NCORES)], axis=0)
    d2 = np.concatenate([res.results[c]["d2_out"] for c in range(NCORES)], axis=0)
    idx = np.concatenate([res.results[c]["idx_out"] for c in range(NCORES)], axis=0)

    emb = emb.reshape(B, N, D)
    dist = dist.reshape(B, N, K)
    d2 = d2.reshape(B, N, K)
    idx = idx.reshape(B, N, K)

    # Tie-order fixup: the device selects by d^2; the reference sorts by
    # dist = sqrt(d^2+1e-6), breaking ties by lower index. Two distinct d^2
    # can round to the same f32 dist - reorder indices inside equal-dist
    # runs to ascending, matching jax.lax.top_k.
    dist_h = np.sqrt(d2 + np.float32(EPS_DIST), dtype=np.float32)
    ties = dist_h[:, :, 1:] == dist_h[:, :, :-1]
    if ties.any():
        rows = np.argwhere(ties.any(axis=2))
        valid = atom_mask > 0
        for bb, nn_ in rows:
            if not valid[bb, nn_]:
                continue
            row_d = dist_h[bb, nn_]
            row_i = idx[bb, nn_]
            s = 0
            while s < K:
                e = s + 1
                while e < K and row_d[e] == row_d[s]:
                    e += 1
                if e - s > 1:
                    row_i[s:e] = np.sort(row_i[s:e])
                s = e
            idx[bb, nn_] = row_i

    return emb, dist, idx.astype(np.int32)
